# revision 1
# baseline (speedup 1.0000x reference)
"""NodeClsPooler: out = x[first_node_of_each_graph] @ W.T + b, distributed over 8 NeuronCores.

Contract: kernel(**inputs) takes FULL inputs (x [1048576,128] f32, batch [1048576] int,
W [128,128] f32, b [128] f32) and returns the FULL [8192,128] f32 output.

Strategy (data-parallel over graphs, 1024 graphs per core):
  - host: first-node index per graph via searchsorted on the sorted batch vector,
    gather those 8192 rows (4MB — the only part of x the op reads), transpose to
    channel-major, shard contiguously across the 8 cores
  - device (SPMD): out_t = W @ pooled_t + b on the TensorEngine; bias is added as a
    rank-1 accumulating matmul (b outer ones) into the same PSUM tile
  - host: concat core outputs and transpose back
"""

import numpy as np

NUM_GRAPHS = 8192
C = 128
N_CORES = 8
G_PER = NUM_GRAPHS // N_CORES  # 1024 graphs per core
FREE = 512  # PSUM bank: 2KB/partition = 512 f32

_CACHE: dict = {}


def _build_program():
    import concourse.bass as bass
    import concourse.mybir as mybir
    import concourse.tile as tile
    from concourse import bacc

    f32 = mybir.dt.float32
    nc = bacc.Bacc("TRN2", target_bir_lowering=False, debug=False)

    pt_d = nc.dram_tensor("pt", [C, G_PER], f32, kind="ExternalInput").ap()
    wt_d = nc.dram_tensor("wt", [C, C], f32, kind="ExternalInput").ap()
    b_d = nc.dram_tensor("brow", [1, C], f32, kind="ExternalInput").ap()
    out_d = nc.dram_tensor("out_t", [C, G_PER], f32, kind="ExternalOutput").ap()

    with tile.TileContext(nc) as tc:
        with (
            tc.tile_pool(name="cst", bufs=1) as cst,
            tc.tile_pool(name="io", bufs=2) as io,
            tc.tile_pool(name="ps", bufs=2, space=bass.MemorySpace.PSUM) as ps,
        ):
            wt_s = cst.tile([C, C], f32)
            nc.sync.dma_start(out=wt_s[:], in_=wt_d[:])
            b_s = cst.tile([1, C], f32)
            nc.sync.dma_start(out=b_s[:], in_=b_d[:])
            ones_s = cst.tile([1, FREE], f32)
            nc.vector.memset(ones_s[:], 1.0)

            for j in range(G_PER // FREE):
                sl = slice(j * FREE, (j + 1) * FREE)
                pt_s = io.tile([C, FREE], f32)
                nc.sync.dma_start(out=pt_s[:], in_=pt_d[:, sl])
                acc = ps.tile([C, FREE], f32)
                # acc = wt_s.T @ pt_s = W @ pooled_t  (out-channel-major)
                nc.tensor.matmul(acc[:], wt_s[:], pt_s[:], start=True, stop=False)
                # acc += outer(b, ones) — broadcast bias along the graph axis
                nc.tensor.matmul(
                    acc[:], b_s[:], ones_s[:], start=False, stop=True,
                    skip_group_check=True,
                )
                o_s = io.tile([C, FREE], f32)
                nc.scalar.copy(out=o_s[:], in_=acc[:])
                nc.sync.dma_start(out=out_d[:, sl], in_=o_s[:])

    nc.compile()
    return nc


def _get_program():
    if "nc" not in _CACHE:
        _CACHE["nc"] = _build_program()
    return _CACHE["nc"]


def kernel(x, batch, W, b, _trace=False, _trace_kwargs=None):
    from concourse.bass_utils import run_bass_kernel_spmd

    x = np.asarray(x)
    batch = np.asarray(batch)
    W = np.ascontiguousarray(np.asarray(W, dtype=np.float32))
    b = np.asarray(b, dtype=np.float32)

    # First occurrence of each graph id in the sorted batch vector (== jnp.searchsorted
    # side='left'); clamp like jnp gather does for any graph id past the last node.
    first = np.searchsorted(batch, np.arange(NUM_GRAPHS, dtype=batch.dtype))
    first = np.minimum(first, x.shape[0] - 1)
    pooled_t = np.ascontiguousarray(x[first].T)  # [C, NUM_GRAPHS] channel-major

    wt = np.ascontiguousarray(W.T)
    brow = np.ascontiguousarray(b.reshape(1, C))
    in_maps = [
        {
            "pt": np.ascontiguousarray(pooled_t[:, k * G_PER : (k + 1) * G_PER]),
            "wt": wt,
            "brow": brow,
        }
        for k in range(N_CORES)
    ]

    nc = _get_program()
    res = run_bass_kernel_spmd(
        nc, in_maps, list(range(N_CORES)),
        trace=_trace, **(_trace_kwargs or {}),
    )
    out_t = np.concatenate(
        [res.results[k]["out_t"] for k in range(N_CORES)], axis=1
    )  # [C, NUM_GRAPHS]
    out = np.ascontiguousarray(out_t.T, dtype=np.float32)  # [NUM_GRAPHS, C]
    if _trace:
        _CACHE["last_results"] = res
    return out


# revision 3
# speedup vs baseline: 1.0710x; 1.0710x over previous
"""NodeClsPooler: out = x[first_node_of_each_graph] @ W.T + b, distributed over 8 NeuronCores.

Contract: kernel(**inputs) takes FULL inputs (x [1048576,128] f32, batch [1048576] int,
W [128,128] f32, b [128] f32) and returns the FULL [8192,128] f32 output.

Strategy (data-parallel over graphs, 1024 graphs per core):
  - host: first-node index per graph via searchsorted on the sorted batch vector,
    gather those 8192 rows (4MB — the only part of x the op reads), transpose to
    channel-major, shard contiguously across the 8 cores
  - device (SPMD): out_t = W @ pooled_t + b; matmul on the TensorEngine in
    out-channel-major orientation, bias added by the DVE during the PSUM->SBUF copy
    (tensor_scalar_add with a per-partition scalar)
  - host: concat core outputs and transpose back
"""

import numpy as np

NUM_GRAPHS = 8192
C = 128
N_CORES = 8
G_PER = NUM_GRAPHS // N_CORES  # 1024 graphs per core
FREE = 512  # PSUM bank: 2KB/partition = 512 f32

_CACHE: dict = {}


def _build_program(mm_dtype_name="float32"):
    import concourse.bass as bass
    import concourse.mybir as mybir
    import concourse.tile as tile
    from concourse import bacc

    f32 = mybir.dt.float32
    mmdt = getattr(mybir.dt, mm_dtype_name)
    nc = bacc.Bacc("TRN2", target_bir_lowering=False, debug=False)

    pt_d = nc.dram_tensor("pt", [C, G_PER], mmdt, kind="ExternalInput").ap()
    wt_d = nc.dram_tensor("wt", [C, C], mmdt, kind="ExternalInput").ap()
    b_d = nc.dram_tensor("bcol", [C, 1], f32, kind="ExternalInput").ap()
    out_d = nc.dram_tensor("out_t", [C, G_PER], f32, kind="ExternalOutput").ap()

    with tile.TileContext(nc) as tc:
        with (
            tc.tile_pool(name="cst", bufs=1) as cst,
            tc.tile_pool(name="io", bufs=2) as io,
            tc.tile_pool(name="ps", bufs=2, space=bass.MemorySpace.PSUM) as ps,
        ):
            wt_s = cst.tile([C, C], mmdt)
            nc.gpsimd.dma_start(out=wt_s[:], in_=wt_d[:])
            b_s = cst.tile([C, 1], f32)
            nc.scalar.dma_start(out=b_s[:], in_=b_d[:])

            for j in range(G_PER // FREE):
                sl = slice(j * FREE, (j + 1) * FREE)
                pt_s = io.tile([C, FREE], mmdt)
                nc.sync.dma_start(out=pt_s[:], in_=pt_d[:, sl])
                acc = ps.tile([C, FREE], f32)
                # acc = wt_s.T @ pt_s = W @ pooled_t  (out-channel-major)
                nc.tensor.matmul(acc[:], wt_s[:], pt_s[:], start=True, stop=True)
                o_s = io.tile([C, FREE], f32)
                # o = acc + b (per-partition scalar broadcast along graphs)
                nc.vector.tensor_scalar_add(o_s[:], acc[:], b_s[:])
                nc.scalar.dma_start(out=out_d[:, sl], in_=o_s[:])

    nc.compile()
    return nc


def _get_program():
    if "nc" not in _CACHE:
        _CACHE["nc"] = _build_program(_CACHE.get("mm_dtype", "float32"))
    return _CACHE["nc"]


def kernel(x, batch, W, b, _trace=False, _trace_kwargs=None):
    from concourse.bass_utils import run_bass_kernel_spmd

    x = np.asarray(x)
    batch = np.asarray(batch)
    W = np.ascontiguousarray(np.asarray(W, dtype=np.float32))
    b = np.asarray(b, dtype=np.float32)

    # First occurrence of each graph id in the sorted batch vector (== jnp.searchsorted
    # side='left'); clamp like jnp gather does for any graph id past the last node.
    first = np.searchsorted(batch, np.arange(NUM_GRAPHS, dtype=batch.dtype))
    first = np.minimum(first, x.shape[0] - 1)
    pooled_t = np.ascontiguousarray(x[first].T)  # [C, NUM_GRAPHS] channel-major

    wt = np.ascontiguousarray(W.T)
    bcol = np.ascontiguousarray(b.reshape(C, 1))
    in_maps = [
        {
            "pt": np.ascontiguousarray(pooled_t[:, k * G_PER : (k + 1) * G_PER]),
            "wt": wt,
            "bcol": bcol,
        }
        for k in range(N_CORES)
    ]

    nc = _get_program()
    res = run_bass_kernel_spmd(
        nc, in_maps, list(range(N_CORES)),
        trace=_trace, **(_trace_kwargs or {}),
    )
    out_t = np.concatenate(
        [res.results[k]["out_t"] for k in range(N_CORES)], axis=1
    )  # [C, NUM_GRAPHS]
    out = np.ascontiguousarray(out_t.T, dtype=np.float32)  # [NUM_GRAPHS, C]
    if _trace:
        _CACHE["last_results"] = res
    return out


# revision 4
# speedup vs baseline: 1.3788x; 1.2875x over previous
"""NodeClsPooler: out = x[first_node_of_each_graph] @ W.T + b, distributed over 8 NeuronCores.

Contract: kernel(**inputs) takes FULL inputs (x [1048576,128] f32, batch [1048576] int,
W [128,128] f32, b [128] f32) and returns the FULL [8192,128] f32 output.

Strategy (data-parallel over graphs, 1024 graphs per core):
  - host: first-node index per graph via searchsorted on the sorted batch vector,
    gather those 8192 rows (4MB — the only part of x the op reads), transpose to
    channel-major, shard contiguously across the 8 cores
  - device (SPMD, raw Bass, hand-scheduled): out_t = W @ pooled_t + b.
    Input DMAs are ordered by criticality across the three DMA-capable engines'
    queues (wt first — it gates matmul0). Matmul chunks are uneven [512,384,128]
    so the large output transfers overlap later matmuls and the final chunk's
    copy+DMA tail is small. fp32 matmuls (exact); bias added by the DVE during
    the PSUM->SBUF copy (tensor_scalar_add with per-partition scalar). No
    end-of-kernel DMA completion waits — the runtime drains DGE queues at NEFF
    exit.
  - host: concat core outputs and transpose back
"""

import numpy as np

NUM_GRAPHS = 8192
C = 128
N_CORES = 8
G_PER = NUM_GRAPHS // N_CORES  # 1024 graphs per core
CH = [(0, 512), (512, 896), (896, 1024)]  # matmul chunks (columns of pooled_t shard)

_CACHE: dict = {}


def _build_program():
    import contextlib

    import concourse.bass as bass
    import concourse.mybir as mybir

    f32 = mybir.dt.float32
    nc = bass.Bass(target_bir_lowering=False, debug=False)

    pt_d = nc.dram_tensor("pt", [C, G_PER], f32, kind="ExternalInput").ap()
    wt_d = nc.dram_tensor("wt", [C, C], f32, kind="ExternalInput").ap()
    b_d = nc.dram_tensor("bcol", [C, 1], f32, kind="ExternalInput").ap()
    out_d = nc.dram_tensor("out_t", [C, G_PER], f32, kind="ExternalOutput").ap()

    sem_names = [
        "wsem", "bsem", "m0", "m1", "m2", "v0", "v1", "v2", "o0", "o1", "o2",
        "pA", "pB", "pC", "pD", "pE",
    ]

    with contextlib.ExitStack() as es:
        sem = {n: es.enter_context(nc.semaphore(n)) for n in sem_names}
        wt_s = es.enter_context(nc.sbuf_tensor("wt_s", [C, C], f32)).ap()
        b_s = es.enter_context(nc.sbuf_tensor("b_s", [C, 1], f32)).ap()
        pt_s = es.enter_context(nc.sbuf_tensor("pt_s", [C, G_PER], f32)).ap()
        acc = [
            es.enter_context(nc.psum_tensor(f"acc{k}", [C, hi - lo], f32)).ap()
            for k, (lo, hi) in enumerate(CH)
        ]
        o_s = es.enter_context(nc.sbuf_tensor("o_s", [C, G_PER], f32)).ap()

        with nc.Block() as block:

            @block.sync
            def _(sync):
                sync.dma_start(out=wt_s, in_=wt_d).then_inc(sem["wsem"], 16)
                sync.dma_start(out=pt_s[:, 512:768], in_=pt_d[:, 512:768]).then_inc(
                    sem["pC"], 16
                )
                sync.dma_start(out=b_s, in_=b_d).then_inc(sem["bsem"], 16)
                sync.wait_ge(sem["v2"], 1)
                sync.dma_start(out=out_d[:, 896:], in_=o_s[:, 896:]).then_inc(
                    sem["o2"], 16
                )

            @block.scalar
            def _(s):
                s.dma_start(out=pt_s[:, 0:256], in_=pt_d[:, 0:256]).then_inc(
                    sem["pA"], 16
                )
                s.dma_start(out=pt_s[:, 768:896], in_=pt_d[:, 768:896]).then_inc(
                    sem["pD"], 16
                )
                s.wait_ge(sem["v1"], 1)
                s.dma_start(out=out_d[:, 512:896], in_=o_s[:, 512:896]).then_inc(
                    sem["o1"], 16
                )

            @block.gpsimd
            def _(g):
                g.dma_start(out=pt_s[:, 256:512], in_=pt_d[:, 256:512]).then_inc(
                    sem["pB"], 16
                )
                g.dma_start(out=pt_s[:, 896:], in_=pt_d[:, 896:]).then_inc(
                    sem["pE"], 16
                )
                g.wait_ge(sem["v0"], 1)
                g.dma_start(out=out_d[:, 0:512], in_=o_s[:, 0:512]).then_inc(
                    sem["o0"], 16
                )

            @block.tensor
            def _(t):
                t.wait_ge(sem["wsem"], 16)
                needs = [["pA", "pB"], ["pC", "pD"], ["pE"]]
                for k, (lo, hi) in enumerate(CH):
                    for n in needs[k]:
                        t.wait_ge(sem[n], 16)
                    t.matmul(
                        acc[k], wt_s, pt_s[:, lo:hi], start=True, stop=True
                    ).then_inc(sem[f"m{k}"], 1)

            @block.vector
            def _(v):
                v.wait_ge(sem["bsem"], 16)
                for k, (lo, hi) in enumerate(CH):
                    v.wait_ge(sem[f"m{k}"], 1)
                    v.tensor_scalar_add(o_s[:, lo:hi], acc[k], b_s).then_inc(
                        sem[f"v{k}"], 1
                    )

    return nc


def _get_program():
    if "nc" not in _CACHE:
        _CACHE["nc"] = _build_program()
    return _CACHE["nc"]


def kernel(x, batch, W, b, _trace=False, _trace_kwargs=None):
    from concourse.bass_utils import run_bass_kernel_spmd

    x = np.asarray(x)
    batch = np.asarray(batch)
    W = np.ascontiguousarray(np.asarray(W, dtype=np.float32))
    b = np.asarray(b, dtype=np.float32)

    # First occurrence of each graph id in the sorted batch vector (== jnp.searchsorted
    # side='left'); clamp like jnp gather does for any graph id past the last node.
    first = np.searchsorted(batch, np.arange(NUM_GRAPHS, dtype=batch.dtype))
    first = np.minimum(first, x.shape[0] - 1)
    pooled_t = np.ascontiguousarray(x[first].T)  # [C, NUM_GRAPHS] channel-major

    wt = np.ascontiguousarray(W.T)
    bcol = np.ascontiguousarray(b.reshape(C, 1))
    in_maps = [
        {
            "pt": np.ascontiguousarray(pooled_t[:, k * G_PER : (k + 1) * G_PER]),
            "wt": wt,
            "bcol": bcol,
        }
        for k in range(N_CORES)
    ]

    nc = _get_program()
    res = run_bass_kernel_spmd(
        nc, in_maps, list(range(N_CORES)),
        trace=_trace, **(_trace_kwargs or {}),
    )
    out_t = np.concatenate(
        [res.results[k]["out_t"] for k in range(N_CORES)], axis=1
    )  # [C, NUM_GRAPHS]
    out = np.ascontiguousarray(out_t.T, dtype=np.float32)  # [NUM_GRAPHS, C]
    if _trace:
        _CACHE["last_results"] = res
    return out
